# revision 17
# baseline (speedup 1.0000x reference)
"""ConVIRT loss (NT-Xent both directions) on 8 Trainium2 NeuronCores.

Sharding: 2D decomposition of the NxN sim matrix, 4 img-row blocks x 2
text-row blocks.  Core (a, b) handles img rows [a*2048, (a+1)*2048) x
text rows [b*4096, (b+1)*4096).

All O(N*D) prep runs on the HOST inside kernel(): normalize rows (f32),
compute the diagonal sim_ii/TEMP (f64), transpose both modality blocks to
d-major [KC, 128, rows] layout, and cast to the matmul operand dtype
(bf16, or fp8e4m3 pre-scaled by 16 to dodge subnormals).  The device then
runs a pure GEMM pipeline with zero PE transposes:

  per text row-tile jt (stationary, 128 rows):
    PE:  psum[jt-tile, i-chunk] += textT_k.T @ imgT_k   (k outer, so the 4
         (bf16) / 2 (fp8 DoubleRow) stationary loads per jt amortize over
         the 4 moving sweeps each)
    ACT: e = exp(psum * scale)  -> SBUF bf16, accum_out -> colsum partial
    DVE/GPSIMD: racc[chunk] += e  (rowsum partials, two independent
         chains split across both engines; final partition-reduce via
         f32 ones-matmuls at the end)

Host combines: rowsum/colsum partials summed across cores, logs, ALPHA
blend with the host-side diagonal.
"""

import math
import os
import numpy as np
import ml_dtypes

import concourse.bacc as bacc
import concourse.tile as tile
import concourse.mybir as mybir
from concourse.bass_utils import run_bass_kernel_spmd

N, D = 8192, 512
CORES = 8
GA, GB = 4, 2                 # img blocks x text blocks
BI = N // GA                  # 2048 img rows per core
BT = N // GB                  # 4096 text rows per core
NTT = BT // 128               # 32 stationary text tiles
KC = D // 128                 # 4 contraction chunks of 128
PW = int(os.environ.get("PW", "1024"))   # psum tile width (2 banks default)
NP = BI // PW                 # psum tiles per jt
TEMP, ALPHA, EPS = 0.1, 0.75, 1e-8
FP8_SCALE = 16.0              # pre-scale fp8 operands out of subnormals

f32 = mybir.dt.float32
bf16 = mybir.dt.bfloat16
fp8 = mybir.dt.float8e4
AF = mybir.ActivationFunctionType
ALU = mybir.AluOpType
AX = mybir.AxisListType

_CACHE = {}


def _cfg():
    return dict(
        fp8=os.environ.get("FP8", "1") == "1",
        repeat=int(os.environ.get("REPEAT", "1")),
        chj=int(os.environ.get("CHJ", "8")),      # text DMA chunk, in jt units
        psb=int(os.environ.get("PSB", "6" if PW == 512 else "3")),
        eb=int(os.environ.get("EB", "3")),        # e pool bufs
        racc=os.environ.get("RACC", "dve"),       # add engine: dve|split|gp
        rdt=os.environ.get("RDT", "bf16"),        # racc accumulator dtype
        csm=os.environ.get("CSM", "add"),         # colsum via: add|act
        bisect=os.environ.get("BISECT", ""),
    )


def _build(cfg=None):
    cfg = cfg or _cfg()
    use_fp8 = cfg["fp8"]
    mmdt = fp8 if use_fp8 else bf16
    act_scale = (1.0 / TEMP) / (FP8_SCALE * FP8_SCALE) if use_fp8 else 1.0 / TEMP
    KG = KC // 2 if use_fp8 else KC               # stationary loads per (jt, sweep)
    import contextlib

    nc = bacc.Bacc("TRN2", target_bir_lowering=False, debug=False)

    z_imgT = nc.dram_tensor("z_imgT", [KC, 128, BI], mmdt, kind="ExternalInput")
    z_textT = nc.dram_tensor("z_textT", [KC, 128, BT], mmdt, kind="ExternalInput")
    out_rowsum = nc.dram_tensor("out_rowsum", [1, BI], f32,
                                kind="ExternalOutput")
    out_colsum = nc.dram_tensor("out_colsum", [128, NTT, NP], f32,
                                kind="ExternalOutput")

    with tile.TileContext(nc) as tc:
        with (
            tc.tile_pool(name="pers", bufs=1) as pers,
            tc.tile_pool(name="e", bufs=cfg["eb"]) as epool,
            tc.tile_pool(name="ps", bufs=cfg["psb"], space="PSUM") as pspool,
            tc.tile_pool(name="psr", bufs=1, space="PSUM") as psrpool,
        ):
            rdt = bf16 if cfg["rdt"] == "bf16" else f32
            ones = pers.tile([128, 1], rdt, tag="ones")
            nc.vector.memset(ones[:], 1.0)

            imgT = pers.tile([128, KC, BI], mmdt, tag="imgT")
            textT = pers.tile([128, KC, BT], mmdt, tag="textT")
            racc = pers.tile([128, NP, PW], rdt, tag="racc")
            csacc = pers.tile([128, NTT, NP], f32, tag="csacc")
            rs = pers.tile([1, BI], f32, tag="rs")

            loop_cm = (tc.For_i(0, cfg["repeat"], 1) if cfg["repeat"] > 1
                       else contextlib.nullcontext())
            with loop_cm:
                nc.sync.dma_start(imgT[:], z_imgT.rearrange("k p b -> p k b"))

                chj = cfg["chj"]
                for g in range(NTT // chj):
                    sl = slice(g * chj * 128, (g + 1) * chj * 128)
                    nc.sync.dma_start(
                        textT[:, :, sl],
                        z_textT[:, :, sl].rearrange("k p b -> p k b"))

                for jt in range(NTT if cfg["bisect"] != "dmaonly" else 0):
                    ps = [pspool.tile([128, PW], f32, tag="ps",
                                      name=f"ps{t}_{jt}")
                          for t in range(NP)]
                    for kk in range(KG):
                        if use_fp8:
                            lhs = textT[:, 2 * kk:2 * kk + 2,
                                        jt * 128:(jt + 1) * 128]
                        else:
                            lhs = textT[:, kk, jt * 128:(jt + 1) * 128]
                        for c in range(BI // 512):
                            t, h = divmod(c, PW // 512)
                            if use_fp8:
                                nc.tensor.matmul(
                                    ps[t][:, h * 512:(h + 1) * 512],
                                    lhs,
                                    imgT[:, 2 * kk:2 * kk + 2,
                                         c * 512:(c + 1) * 512],
                                    start=(kk == 0), stop=(kk == KG - 1),
                                    perf_mode=mybir.MatmulPerfMode.DoubleRow)
                            else:
                                nc.tensor.matmul(
                                    ps[t][:, h * 512:(h + 1) * 512],
                                    lhs,
                                    imgT[:, kk, c * 512:(c + 1) * 512],
                                    start=(kk == 0), stop=(kk == KG - 1))
                    if cfg["bisect"] == "noexp":
                        continue
                    for t in range(NP):
                        e = epool.tile([128, PW], bf16, tag="e",
                                        name=f"e{t}_{jt}")
                        act_acc = (cfg["csm"] == "act"
                                   or (cfg["csm"] == "mix" and t == 1))
                        if act_acc:
                            nc.scalar.activation(
                                e[:], ps[t][:], AF.Exp, scale=act_scale,
                                accum_out=csacc[:, jt, t:t + 1])
                        else:
                            nc.scalar.activation(
                                e[:], ps[t][:], AF.Exp, scale=act_scale)
                        if cfg["csm"] == "mix":
                            eng = nc.vector if t == 0 else nc.gpsimd
                        else:
                            eng = {"split": (nc.vector if t % 2 == 0
                                             else nc.gpsimd),
                                   "dve": nc.vector,
                                   "gp": nc.gpsimd}[cfg["racc"]]
                        if act_acc:
                            if jt == 0:
                                eng.tensor_scalar(
                                    racc[:, t, :], e[:], 1.0, 0.0,
                                    op0=ALU.mult, op1=ALU.add)
                            else:
                                eng.tensor_tensor(
                                    racc[:, t, :], racc[:, t, :], e[:],
                                    op=ALU.add)
                        elif jt == 0:
                            eng.tensor_scalar(
                                racc[:, t, :], e[:], 1.0, 0.0, op0=ALU.mult,
                                op1=ALU.add,
                                accum_out=csacc[:, jt, t:t + 1])
                        else:
                            eng.scalar_tensor_tensor(
                                racc[:, t, :], e[:], 1.0, racc[:, t, :],
                                op0=ALU.mult, op1=ALU.add,
                                accum_out=csacc[:, jt, t:t + 1])

                if cfg["bisect"] in ("noexp", "dmaonly"):
                    nc.vector.memset(rs[:], 1.0)
                    nc.vector.memset(csacc[:], 1.0)
                else:
                    for c in range(BI // 512):
                        t, h = divmod(c, PW // 512)
                        psr = psrpool.tile([1, 512], f32, tag="psr",
                                           name=f"psr{c}")
                        nc.tensor.matmul(
                            psr[:], ones[:],
                            racc[:, t, h * 512:(h + 1) * 512],
                            start=True, stop=True)
                        if c % 2 == 0:
                            nc.scalar.copy(rs[:, c * 512:(c + 1) * 512], psr[:])
                        else:
                            nc.vector.tensor_copy(
                                rs[:, c * 512:(c + 1) * 512], psr[:])
                nc.sync.dma_start(out_rowsum[:], rs[:])
                nc.sync.dma_start(out_colsum[:], csacc[:])

    nc.compile()
    return nc


def get_program():
    key = tuple(sorted(_cfg().items()))
    if key not in _CACHE:
        _CACHE[key] = _build()
    return _CACHE[key]


def core_block(c):
    """Core c -> (img block a, text block b)."""
    return c % GA, c // GA


def _host_prep(z_img, z_text):
    """Normalize (f32), diag (f64), transpose to [KC, 128, N] operand dtype."""
    use_fp8 = _cfg()["fp8"]
    zi = np.ascontiguousarray(z_img, dtype=np.float32)
    zt = np.ascontiguousarray(z_text, dtype=np.float32)
    ni = np.maximum(np.sqrt(np.einsum("nd,nd->n", zi, zi)), EPS)
    nt = np.maximum(np.sqrt(np.einsum("nd,nd->n", zt, zt)), EPS)
    zi_n = zi / ni[:, None]
    zt_n = zt / nt[:, None]
    diag = np.einsum("nd,nd->n", zi_n.astype(np.float64),
                     zt_n.astype(np.float64)) / TEMP
    if use_fp8:
        dt = mybir.dt.np(fp8)
        ziT = (zi_n.T * FP8_SCALE).astype(dt).reshape(KC, 128, N)
        ztT = (zt_n.T * FP8_SCALE).astype(dt).reshape(KC, 128, N)
    else:
        ziT = zi_n.T.astype(ml_dtypes.bfloat16).reshape(KC, 128, N)
        ztT = zt_n.T.astype(ml_dtypes.bfloat16).reshape(KC, 128, N)
    return ziT, ztT, diag


def make_in_maps(z_img, z_text):
    ziT, ztT, diag = _host_prep(z_img, z_text)
    maps = []
    for c in range(CORES):
        a, b = core_block(c)
        maps.append({
            "z_imgT": np.ascontiguousarray(ziT[:, :, a * BI:(a + 1) * BI]),
            "z_textT": np.ascontiguousarray(ztT[:, :, b * BT:(b + 1) * BT]),
        })
    return maps, diag


def _colsum_rows(cs):
    """Device out_colsum [128, NTT, NP] -> per-text-row colsum [BT]."""
    cs = np.asarray(cs, np.float64).copy()
    mode = _cfg()["csm"]
    for t in range(NP):
        if mode == "add" or (mode == "mix" and t == 0):
            # running sums: per-tile sums are consecutive diffs
            cs[:, 1:, t] = np.diff(cs[:, :, t], axis=1)
    return cs.sum(axis=2).T.reshape(-1)


def combine(results, diag):
    rows = np.zeros(N, np.float64)
    cols = np.zeros(N, np.float64)
    for c in range(CORES):
        a, b = core_block(c)
        rows[a * BI:(a + 1) * BI] += np.asarray(
            results[c]["out_rowsum"], np.float64).reshape(-1)
        cols[b * BT:(b + 1) * BT] += _colsum_rows(results[c]["out_colsum"])
    loss_a = np.mean(np.log(rows) - diag)
    loss_b = np.mean(np.log(cols) - diag)
    return np.float32(ALPHA * loss_a + (1.0 - ALPHA) * loss_b)


def _run_sim(nc, maps):
    from concourse.bass_interp import CoreSim
    outs = []
    for m in maps:
        sim = CoreSim(nc, trace=False)
        for k, v in m.items():
            sim.tensor(k)[:] = v
        sim.simulate()
        outs.append({n: np.array(sim.tensor(n))
                     for n in ("out_rowsum", "out_colsum")})
    return outs


def kernel(z_img, z_text):
    nc = get_program()
    maps, diag = make_in_maps(z_img, z_text)
    try:
        res = run_bass_kernel_spmd(nc, maps, list(range(CORES))).results
    except Exception:
        res = _run_sim(nc, maps)
    return combine(res, diag)


if __name__ == "__main__":
    rng = np.random.default_rng(0)
    out = kernel(rng.standard_normal((N, D), dtype=np.float32),
                 rng.standard_normal((N, D), dtype=np.float32))
    print("loss:", out)


# revision 18
# speedup vs baseline: 1.3406x; 1.3406x over previous
"""ConVIRT loss (NT-Xent both directions) on 8 Trainium2 NeuronCores.

Sharding: 2D decomposition of the NxN sim matrix, 4 img-row blocks x 2
text-row blocks.  Core (a, b) handles img rows [a*2048, (a+1)*2048) x
text rows [b*4096, (b+1)*4096).

All O(N*D) prep runs on the HOST inside kernel(): normalize rows (f32),
compute the diagonal sim_ii/TEMP (f64), transpose both modality blocks to
d-major [KC, 128, rows] layout, and cast to the matmul operand dtype
(bf16, or fp8e4m3 pre-scaled by 16 to dodge subnormals).  The device then
runs a pure GEMM pipeline with zero PE transposes:

  per text row-tile jt (stationary, 128 rows):
    PE:  psum[jt-tile, i-chunk] += textT_k.T @ imgT_k   (k outer, so the 4
         (bf16) / 2 (fp8 DoubleRow) stationary loads per jt amortize over
         the 4 moving sweeps each)
    ACT: e = exp(psum * scale)  -> SBUF bf16, accum_out -> colsum partial
    DVE/GPSIMD: racc[chunk] += e  (rowsum partials, two independent
         chains split across both engines; final partition-reduce via
         f32 ones-matmuls at the end)

Host combines: rowsum/colsum partials summed across cores, logs, ALPHA
blend with the host-side diagonal.
"""

import math
import os
import numpy as np
import ml_dtypes

import concourse.bacc as bacc
import concourse.tile as tile
import concourse.mybir as mybir
from concourse.bass_utils import run_bass_kernel_spmd

N, D = 8192, 512
CORES = 8
GA, GB = 4, 2                 # img blocks x text blocks
BI = N // GA                  # 2048 img rows per core
BT = N // GB                  # 4096 text rows per core
NTT = BT // 128               # 32 stationary text tiles
KC = D // 128                 # 4 contraction chunks of 128
PW = int(os.environ.get("PW", "1024"))   # psum tile width (2 banks default)
NP = BI // PW                 # psum tiles per jt
TEMP, ALPHA, EPS = 0.1, 0.75, 1e-8
FP8_SCALE = 16.0              # pre-scale fp8 operands out of subnormals

f32 = mybir.dt.float32
bf16 = mybir.dt.bfloat16
fp8 = mybir.dt.float8e4
AF = mybir.ActivationFunctionType
ALU = mybir.AluOpType
AX = mybir.AxisListType

_CACHE = {}


def _cfg():
    return dict(
        fp8=os.environ.get("FP8", "1") == "1",
        repeat=int(os.environ.get("REPEAT", "1")),
        chj=int(os.environ.get("CHJ", "8")),      # text DMA chunk, in jt units
        psb=int(os.environ.get("PSB", "6" if PW == 512 else "3")),
        eb=int(os.environ.get("EB", "3")),        # e pool bufs
        racc=os.environ.get("RACC", "dve"),       # add engine: dve|split|gp
        rdt=os.environ.get("RDT", "bf16"),        # racc accumulator dtype
        csm=os.environ.get("CSM", "add"),         # colsum via: add|act
        bisect=os.environ.get("BISECT", ""),
    )


def _build(cfg=None):
    cfg = cfg or _cfg()
    use_fp8 = cfg["fp8"]
    mmdt = fp8 if use_fp8 else bf16
    act_scale = (1.0 / TEMP) / (FP8_SCALE * FP8_SCALE) if use_fp8 else 1.0 / TEMP
    KG = KC // 2 if use_fp8 else KC               # stationary loads per (jt, sweep)
    import contextlib

    nc = bacc.Bacc("TRN2", target_bir_lowering=False, debug=False)

    z_imgT = nc.dram_tensor("z_imgT", [KC, 128, BI], mmdt, kind="ExternalInput")
    z_textT = nc.dram_tensor("z_textT", [KC, 128, BT], mmdt, kind="ExternalInput")
    out_rowsum = nc.dram_tensor("out_rowsum", [1, BI], f32,
                                kind="ExternalOutput")
    out_colsum = nc.dram_tensor("out_colsum", [128, NTT, NP], f32,
                                kind="ExternalOutput")

    with tile.TileContext(nc) as tc:
        with (
            tc.tile_pool(name="pers", bufs=1) as pers,
            tc.tile_pool(name="e", bufs=cfg["eb"]) as epool,
            tc.tile_pool(name="ps", bufs=cfg["psb"], space="PSUM") as pspool,
            tc.tile_pool(name="psr", bufs=1, space="PSUM") as psrpool,
        ):
            rdt = bf16 if cfg["rdt"] == "bf16" else f32
            ones = pers.tile([128, 1], rdt, tag="ones")
            nc.vector.memset(ones[:], 1.0)

            imgT = pers.tile([128, KC, BI], mmdt, tag="imgT")
            textT = pers.tile([128, KC, BT], mmdt, tag="textT")
            racc = pers.tile([128, NP, PW], rdt, tag="racc")
            csacc = pers.tile([128, NTT, NP], f32, tag="csacc")
            rs = pers.tile([1, BI], f32, tag="rs")

            loop_cm = (tc.For_i(0, cfg["repeat"], 1) if cfg["repeat"] > 1
                       else contextlib.nullcontext())
            with loop_cm:
                nc.sync.dma_start(imgT[:], z_imgT.rearrange("k p b -> p k b"))

                chj = cfg["chj"]
                for g in range(NTT // chj):
                    sl = slice(g * chj * 128, (g + 1) * chj * 128)
                    nc.sync.dma_start(
                        textT[:, :, sl],
                        z_textT[:, :, sl].rearrange("k p b -> p k b"))

                for jt in range(NTT if cfg["bisect"] != "dmaonly" else 0):
                    ps = [pspool.tile([128, PW], f32, tag="ps",
                                      name=f"ps{t}_{jt}")
                          for t in range(NP)]
                    for kk in range(KG):
                        if use_fp8:
                            lhs = textT[:, 2 * kk:2 * kk + 2,
                                        jt * 128:(jt + 1) * 128]
                        else:
                            lhs = textT[:, kk, jt * 128:(jt + 1) * 128]
                        for c in range(BI // 512):
                            t, h = divmod(c, PW // 512)
                            if use_fp8:
                                nc.tensor.matmul(
                                    ps[t][:, h * 512:(h + 1) * 512],
                                    lhs,
                                    imgT[:, 2 * kk:2 * kk + 2,
                                         c * 512:(c + 1) * 512],
                                    start=(kk == 0), stop=(kk == KG - 1),
                                    perf_mode=mybir.MatmulPerfMode.DoubleRow)
                            else:
                                nc.tensor.matmul(
                                    ps[t][:, h * 512:(h + 1) * 512],
                                    lhs,
                                    imgT[:, kk, c * 512:(c + 1) * 512],
                                    start=(kk == 0), stop=(kk == KG - 1))
                    if cfg["bisect"] == "noexp":
                        continue
                    for t in range(NP):
                        if cfg["bisect"] == "noadd":
                            e = epool.tile([128, PW], bf16, tag="e",
                                           name=f"e{t}_{jt}")
                            nc.scalar.activation(
                                e[:], ps[t][:], AF.Exp, scale=act_scale)
                            continue
                        e = epool.tile([128, PW], bf16, tag="e",
                                        name=f"e{t}_{jt}")
                        act_acc = (cfg["csm"] == "act"
                                   or (cfg["csm"] == "mix" and t == 1))
                        if act_acc:
                            nc.scalar.activation(
                                e[:], ps[t][:], AF.Exp, scale=act_scale,
                                accum_out=csacc[:, jt, t:t + 1])
                        else:
                            nc.scalar.activation(
                                e[:], ps[t][:], AF.Exp, scale=act_scale)
                        if cfg["csm"] == "mix":
                            eng = nc.vector if t == 0 else nc.gpsimd
                        else:
                            eng = {"split": (nc.vector if t % 2 == 0
                                             else nc.gpsimd),
                                   "dve": nc.vector,
                                   "gp": nc.gpsimd}[cfg["racc"]]
                        if act_acc:
                            if jt == 0:
                                eng.tensor_scalar(
                                    racc[:, t, :], e[:], 1.0, 0.0,
                                    op0=ALU.mult, op1=ALU.add)
                            else:
                                eng.tensor_tensor(
                                    racc[:, t, :], racc[:, t, :], e[:],
                                    op=ALU.add)
                        elif jt == 0:
                            eng.tensor_scalar(
                                racc[:, t, :], e[:], 1.0, 0.0, op0=ALU.mult,
                                op1=ALU.add,
                                accum_out=csacc[:, jt, t:t + 1])
                        else:
                            eng.scalar_tensor_tensor(
                                racc[:, t, :], e[:], 1.0, racc[:, t, :],
                                op0=ALU.mult, op1=ALU.add,
                                accum_out=csacc[:, jt, t:t + 1])

                if cfg["bisect"] in ("noexp", "noadd", "dmaonly"):
                    nc.vector.memset(rs[:], 1.0)
                    nc.vector.memset(csacc[:], 1.0)
                else:
                    for c in range(BI // 512):
                        t, h = divmod(c, PW // 512)
                        psr = psrpool.tile([1, 512], f32, tag="psr",
                                           name=f"psr{c}")
                        nc.tensor.matmul(
                            psr[:], ones[:],
                            racc[:, t, h * 512:(h + 1) * 512],
                            start=True, stop=True)
                        if c % 2 == 0:
                            nc.scalar.copy(rs[:, c * 512:(c + 1) * 512], psr[:])
                        else:
                            nc.vector.tensor_copy(
                                rs[:, c * 512:(c + 1) * 512], psr[:])
                nc.sync.dma_start(out_rowsum[:], rs[:])
                nc.sync.dma_start(out_colsum[:], csacc[:])

    nc.compile()
    return nc


def get_program():
    key = tuple(sorted(_cfg().items()))
    if key not in _CACHE:
        _CACHE[key] = _build()
    return _CACHE[key]


def core_block(c):
    """Core c -> (img block a, text block b)."""
    return c % GA, c // GA


def _host_prep(z_img, z_text):
    """Normalize (f32), diag (f64), transpose to [KC, 128, N] operand dtype."""
    use_fp8 = _cfg()["fp8"]
    zi = np.ascontiguousarray(z_img, dtype=np.float32)
    zt = np.ascontiguousarray(z_text, dtype=np.float32)
    ni = np.maximum(np.sqrt(np.einsum("nd,nd->n", zi, zi)), EPS)
    nt = np.maximum(np.sqrt(np.einsum("nd,nd->n", zt, zt)), EPS)
    zi_n = zi / ni[:, None]
    zt_n = zt / nt[:, None]
    diag = np.einsum("nd,nd->n", zi_n.astype(np.float64),
                     zt_n.astype(np.float64)) / TEMP
    if use_fp8:
        dt = mybir.dt.np(fp8)
        ziT = (zi_n.T * FP8_SCALE).astype(dt).reshape(KC, 128, N)
        ztT = (zt_n.T * FP8_SCALE).astype(dt).reshape(KC, 128, N)
    else:
        ziT = zi_n.T.astype(ml_dtypes.bfloat16).reshape(KC, 128, N)
        ztT = zt_n.T.astype(ml_dtypes.bfloat16).reshape(KC, 128, N)
    return ziT, ztT, diag


def make_in_maps(z_img, z_text):
    ziT, ztT, diag = _host_prep(z_img, z_text)
    maps = []
    for c in range(CORES):
        a, b = core_block(c)
        maps.append({
            "z_imgT": np.ascontiguousarray(ziT[:, :, a * BI:(a + 1) * BI]),
            "z_textT": np.ascontiguousarray(ztT[:, :, b * BT:(b + 1) * BT]),
        })
    return maps, diag


def _colsum_rows(cs):
    """Device out_colsum [128, NTT, NP] -> per-text-row colsum [BT]."""
    cs = np.asarray(cs, np.float64).copy()
    mode = _cfg()["csm"]
    for t in range(NP):
        if mode == "add" or (mode == "mix" and t == 0):
            # running sums: per-tile sums are consecutive diffs
            cs[:, 1:, t] = np.diff(cs[:, :, t], axis=1)
    return cs.sum(axis=2).T.reshape(-1)


def combine(results, diag):
    rows = np.zeros(N, np.float64)
    cols = np.zeros(N, np.float64)
    for c in range(CORES):
        a, b = core_block(c)
        rows[a * BI:(a + 1) * BI] += np.asarray(
            results[c]["out_rowsum"], np.float64).reshape(-1)
        cols[b * BT:(b + 1) * BT] += _colsum_rows(results[c]["out_colsum"])
    loss_a = np.mean(np.log(rows) - diag)
    loss_b = np.mean(np.log(cols) - diag)
    return np.float32(ALPHA * loss_a + (1.0 - ALPHA) * loss_b)


def _run_sim(nc, maps):
    from concourse.bass_interp import CoreSim
    outs = []
    for m in maps:
        sim = CoreSim(nc, trace=False)
        for k, v in m.items():
            sim.tensor(k)[:] = v
        sim.simulate()
        outs.append({n: np.array(sim.tensor(n))
                     for n in ("out_rowsum", "out_colsum")})
    return outs


def kernel(z_img, z_text):
    nc = get_program()
    maps, diag = make_in_maps(z_img, z_text)
    try:
        res = run_bass_kernel_spmd(nc, maps, list(range(CORES))).results
    except Exception:
        res = _run_sim(nc, maps)
    return combine(res, diag)


if __name__ == "__main__":
    rng = np.random.default_rng(0)
    out = kernel(rng.standard_normal((N, D), dtype=np.float32),
                 rng.standard_normal((N, D), dtype=np.float32))
    print("loss:", out)
